# revision 1
# baseline (speedup 1.0000x reference)
"""Multi-resolution dense-grid trilinear interpolation (embedding lookup).

Strategy:
  - Host: pack each LOD grid into a "cell table" tab[cell, 0:8] = the 8 corner
    values of the cell whose base corner has flat index `cell` (flat-index
    aliasing + clip matches jnp's OOB clamp semantics exactly).
  - Shard points across 8 NeuronCores (data parallel, tables replicated).
  - Device per core: for each point batch and each level, compute
    xs = pts*(res-1)/2 + (res-1)/2, frac f = xs mod 1, floor = xs - f,
    cell = (cz*res + cy)*res + cx (exact in f32, < 2^24), then gather the 32B
    cell row with one indirect-DMA descriptor per point, and evaluate the
    trilinear lerp chain on the vector engine. Sum over 5 levels.

Scheduling note: the walrus build in this toolchain only allows ONE sync-wait
command per instruction, so the kernel is structured so Tile never needs two:
cross-engine tiles (idx) get unique slots (no recycle waits), pts/acc are
single tiles loaded/stored once, and each gather is preceded by a tiny Pool
"+0" probe on idx so the idx-ready (DVE) wait lands on the probe while the
gather only carries the cells-slot-recycle (DMASW) wait.
"""

import numpy as np

import concourse.bass as bass
import concourse.mybir as mybir
import concourse.tile as tile
from concourse.bass import IndirectOffsetOnAxis
from concourse.bass_utils import run_bass_kernel_spmd
from concourse.tile import add_dep_helper

LODS = [16, 32, 64, 128, 256]
N_PTS = 2_000_000
N_CORES = 8
P = 128

BATCH_FREE = 490  # points per partition per batch
N_BATCHES = 4
PTS_PER_CORE = P * BATCH_FREE * N_BATCHES  # 250_880 (>= 250_000)

F32 = mybir.dt.float32
I32 = mybir.dt.int32
AF = mybir.ActivationFunctionType
ALU = mybir.AluOpType


def pack_cell_table(cb: np.ndarray, res: int) -> np.ndarray:
    """tab[i, dz*4+dy*2+dx] = cb_flat[min(i + dx + dy*res + dz*res^2, n-1)]."""
    flat = np.ascontiguousarray(np.asarray(cb, dtype=np.float32).reshape(-1))
    n = flat.shape[0]
    tab = np.empty((n, 8), dtype=np.float32)
    base = np.arange(n, dtype=np.int64)
    c = 0
    for dz in (0, 1):
        for dy in (0, 1):
            for dx in (0, 1):
                off = dx + dy * res + dz * res * res
                if off == 0:
                    tab[:, c] = flat
                else:
                    idx = base + off
                    np.minimum(idx, n - 1, out=idx)
                    tab[:, c] = flat[idx]
                c += 1
    return tab


def build_module(batch_free: int = BATCH_FREE, n_batches: int = N_BATCHES):
    """Build the single-core Bass program (run SPMD on all 8 cores)."""
    n = batch_free
    npts = P * n * n_batches
    ntot = n * n_batches  # free elems per partition overall
    nc = bass.Bass("TRN2", target_bir_lowering=False, debug=False)

    pts = nc.dram_tensor("pts", [npts, 3], F32, kind="ExternalInput").ap()
    tabs = [
        nc.dram_tensor(f"tab{i}", [r**3, 8], F32, kind="ExternalInput").ap()
        for i, r in enumerate(LODS)
    ]
    out = nc.dram_tensor("out", [npts, 1], F32, kind="ExternalOutput").ap()

    with tile.TileContext(nc) as tc:
        with (
            tc.tile_pool(name="p_io", bufs=1) as io_pool,
            tc.tile_pool(name="p_coord", bufs=2) as coord_pool,
            tc.tile_pool(name="p_idx", bufs=1) as idx_pool,
            tc.tile_pool(name="p_cells", bufs=3) as cells_pool,
            tc.tile_pool(name="p_lerp", bufs=2) as lerp_pool,
        ):
            # one DMA for all points: [P, ntot, 3], per-partition contiguous
            pts_all = io_pool.tile([P, ntot, 3], F32, name="pts_all")
            pts_load = nc.sync.dma_start(
                out=pts_all[:],
                in_=pts[:, :].rearrange("(p n) t -> p n t", p=P),
            )
            acc_all = io_pool.tile([P, ntot], F32, name="acc_all")
            pool_scratch = io_pool.tile(
                [P, n_batches * len(LODS)], I32, name="pool_scratch"
            )
            dve_obs = io_pool.tile(
                [1, n_batches * len(LODS)], I32, name="dve_obs"
            )
            obs_f32 = io_pool.tile([1, 8], F32, name="obs_f32")
            pool_f32 = io_pool.tile(
                [1, 8 * n_batches * len(LODS)], F32, name="pool_f32"
            )
            probe_slot = 0
            pf_slot = 0
            last_col_gathers = []
            probe_insts = []
            cells_readers = []
            pool_obs_hist = []
            CELLS_BUFS = 3

            for b in range(n_batches):
                pts_b = pts_all[:, b * n : (b + 1) * n, :]
                acc_b = acc_all[:, b * n : (b + 1) * n]

                for li, res in enumerate(LODS):
                    s = (res - 1) / 2.0
                    # xs = pts*s + s
                    xs = coord_pool.tile([P, n, 3], F32, name="xs", tag="xs")
                    nc.vector.tensor_scalar(
                        out=xs[:], in0=pts_b, scalar1=s, scalar2=s,
                        op0=ALU.mult, op1=ALU.add,
                    )
                    # floor via int32 trunc-cast roundtrip (xs >= 0), f = xs - floor
                    ci = coord_pool.tile([P, n, 3], I32, name="ci", tag="ci")
                    nc.vector.tensor_copy(out=ci[:], in_=xs[:])
                    flo = coord_pool.tile([P, n, 3], F32, name="flo", tag="flo")
                    nc.vector.tensor_copy(out=flo[:], in_=ci[:])
                    # robust to the DVE converter's rounding mode: if the
                    # cast rounded up (flo > xs), subtract 1 to get floor.
                    # f doubles as scratch for the correction mask.
                    f = coord_pool.tile([P, n, 3], F32, name="f", tag="f")
                    nc.vector.tensor_tensor(
                        out=f[:], in0=flo[:], in1=xs[:], op=ALU.is_gt
                    )
                    nc.vector.tensor_tensor(
                        out=flo[:], in0=flo[:], in1=f[:], op=ALU.subtract
                    )
                    nc.vector.tensor_tensor(
                        out=f[:], in0=xs[:], in1=flo[:], op=ALU.subtract
                    )
                    # cell = (cz*res + cy)*res + cx  (f32 exact, < 2^24)
                    idx_f = coord_pool.tile([P, n], F32, name="idx_f", tag="idx_f")
                    nc.vector.scalar_tensor_tensor(
                        out=idx_f[:], in0=flo[:, :, 2], scalar=float(res),
                        in1=flo[:, :, 1], op0=ALU.mult, op1=ALU.add,
                    )
                    nc.vector.scalar_tensor_tensor(
                        out=idx_f[:], in0=idx_f[:], scalar=float(res),
                        in1=flo[:, :, 0], op0=ALU.mult, op1=ALU.add,
                    )
                    # own slot per (batch, level): no recycle waits
                    idx = idx_pool.tile(
                        [P, n], I32, name=f"idx_{b}_{li}", tag=f"idx_{b}_{li}"
                    )
                    nc.vector.tensor_copy(out=idx[:], in_=idx_f[:])
                    # Pool-engine probe: reads one idx element into a scratch
                    # tile. The probe absorbs the DVE idx-ready wait into the
                    # Pool sequencer clock; the gathers are ordered after it
                    # with sync-free deps, so they need no wait of their own
                    # for idx (walrus here fits only one sync-wait command
                    # per instruction).
                    probe = nc.gpsimd.tensor_scalar(
                        out=pool_scratch[0:1, probe_slot : probe_slot + 1],
                        in0=idx[0:1, 0:1],
                        scalar1=0, scalar2=None, op0=ALU.add,
                    )
                    probe_slot += 1
                    if len(cells_readers) >= CELLS_BUFS:
                        # merge the recycled slot's last-reader tick into the
                        # probe's DVE wait (same sem -> still one condition)
                        add_dep_helper(
                            probe.ins, cells_readers[-CELLS_BUFS].ins, sync=True,
                            reason="probe covers recycled cells slot readers",
                        )
                        # order after the evicted tenant's Pool lane observers
                        # so the first gather's writer-waits are elided
                        for pob in pool_obs_hist[-CELLS_BUFS]:
                            add_dep_helper(
                                probe.ins, pob.ins, sync=False,
                                reason="probe after prev tenant lane observers",
                            )

                    # gather 8-f32 cell rows, one point-column per instruction
                    # (HW only supports one dynamic offset per partition):
                    # cells[p, i, :] = tab[idx[p, i], :]
                    cells = cells_pool.tile([P, n, 8], F32, name="cells")
                    col_gathers = []
                    for i in range(n):
                        g = nc.gpsimd.indirect_dma_start(
                            out=cells[:, i, :],
                            out_offset=None,
                            in_=tabs[li][:],
                            in_offset=IndirectOffsetOnAxis(
                                ap=idx[:, i : i + 1], axis=0
                            ),
                        )
                        add_dep_helper(
                            g.ins, probe.ins, sync=False,
                            reason="gather after idx probe (pool seq order)",
                        )
                        col_gathers.append(g)
                    last_col_gathers = col_gathers
                    probe_insts.append(probe)
                    # DVE observers: one tiny copy per DMASW lane (the last 8
                    # gathers cover all lanes round-robin), so the lerp's
                    # cells-read needs no multi-lane wait of its own. Pool
                    # observers do the same for the Pool sequencer clock so
                    # the slot's next tenant needs no multi-lane wait either.
                    observers = []
                    pool_obs = []
                    for k in range(min(8, n)):
                        col = n - 1 - k
                        ob = nc.vector.tensor_copy(
                            out=obs_f32[0:1, k : k + 1],
                            in_=cells[0:1, col, 0:1],
                        )
                        observers.append(ob)
                        pob = nc.gpsimd.tensor_scalar(
                            out=pool_f32[0:1, pf_slot : pf_slot + 1],
                            in0=cells[0:1, col, 0:1],
                            scalar1=0.0, scalar2=None, op0=ALU.add,
                        )
                        pf_slot += 1
                        pool_obs.append(pob)
                    pool_obs_hist.append(pool_obs)

                    # trilinear lerp chain: 8 -> 4 -> 2 -> 1 (in-place)
                    cr = cells[:].rearrange("p n (j two) -> p n j two", two=2)
                    d1 = lerp_pool.tile([P, n, 4], F32, name="d1", tag="d1")
                    d1_sub = nc.vector.tensor_tensor(
                        out=d1[:], in0=cr[:, :, :, 1], in1=cr[:, :, :, 0],
                        op=ALU.subtract,
                    )
                    for ob in observers:
                        add_dep_helper(
                            d1_sub.ins, ob.ins, sync=False,
                            reason="lerp after lane observers (dve order)",
                        )
                    fx = f[:, :, 0:1].to_broadcast([P, n, 4])
                    nc.vector.tensor_tensor(out=d1[:], in0=d1[:], in1=fx, op=ALU.mult)
                    t1_add = nc.vector.tensor_tensor(
                        out=d1[:], in0=cr[:, :, :, 0], in1=d1[:], op=ALU.add
                    )
                    cells_readers.append(t1_add)

                    tr = d1[:].rearrange("p n (j two) -> p n j two", two=2)
                    d2 = lerp_pool.tile([P, n, 2], F32, name="d2", tag="d2")
                    nc.vector.tensor_tensor(
                        out=d2[:], in0=tr[:, :, :, 1], in1=tr[:, :, :, 0],
                        op=ALU.subtract,
                    )
                    fy = f[:, :, 1:2].to_broadcast([P, n, 2])
                    nc.vector.tensor_tensor(out=d2[:], in0=d2[:], in1=fy, op=ALU.mult)
                    nc.vector.tensor_tensor(
                        out=d2[:], in0=tr[:, :, :, 0], in1=d2[:], op=ALU.add
                    )

                    d3 = lerp_pool.tile([P, n], F32, name="d3", tag="d3")
                    nc.vector.tensor_tensor(
                        out=d3[:], in0=d2[:, :, 1], in1=d2[:, :, 0], op=ALU.subtract
                    )
                    nc.vector.tensor_tensor(
                        out=d3[:], in0=d3[:], in1=f[:, :, 2], op=ALU.mult
                    )
                    if li == 0:
                        nc.vector.tensor_tensor(
                            out=acc_b, in0=d2[:, :, 0], in1=d3[:], op=ALU.add
                        )
                        final_acc = None
                    else:
                        c3 = lerp_pool.tile([P, n], F32, name="c3", tag="c3")
                        nc.vector.tensor_tensor(
                            out=c3[:], in0=d2[:, :, 0], in1=d3[:], op=ALU.add
                        )
                        final_acc = nc.vector.tensor_tensor(
                            out=acc_b, in0=acc_b, in1=c3[:], op=ALU.add
                        )

            # ---- drain-tail flattening (1 sync-wait per instruction limit) --
            # DVE observes the Pool clock by reading all probe outputs; force
            # it before the final acc so the out-DMA's DVE wait covers it.
            probe_obs = nc.vector.tensor_copy(
                out=dve_obs[:], in_=pool_scratch[0:1, :]
            )
            assert final_acc is not None
            add_dep_helper(
                final_acc.ins, probe_obs.ins, sync=False,
                reason="probe observation before final acc",
            )
            # SP observes every proc's final tick directly, one nop (= one
            # sync-wait) per proc, so the tail drain ends up with a single
            # remaining wait (the out-DMA's own lane).
            obs_targets = [pts_load, probe_insts[-1], probe_obs, final_acc]
            obs_targets += pool_obs_hist[-1]
            obs_targets += last_col_gathers[-8:]
            for k, tgt in enumerate(obs_targets):
                sp_nop = nc.sync.nop(nofuse=True, hint=f"obs_{k}")
                add_dep_helper(
                    sp_nop.ins, tgt.ins, sync=True,
                    reason="SP observes proc completion before drain",
                )

            nc.sync.dma_start(
                out=out[:, :].rearrange("(p n) o -> p (n o)", p=P),
                in_=acc_all[:],
            )

    return nc


_MODULE_CACHE = {}


def _get_module():
    key = (BATCH_FREE, N_BATCHES)
    if key not in _MODULE_CACHE:
        _MODULE_CACHE[key] = build_module()
    return _MODULE_CACHE[key]


def kernel(pts, cb0, cb1, cb2, cb3, cb4):
    nc = _get_module()
    cbs = [cb0, cb1, cb2, cb3, cb4]
    tabs = [pack_cell_table(cb, r) for cb, r in zip(cbs, LODS)]

    pts = np.ascontiguousarray(np.asarray(pts, dtype=np.float32))
    assert pts.shape == (N_PTS, 3)
    pts_pad = np.zeros((N_CORES * PTS_PER_CORE, 3), dtype=np.float32)
    pts_pad[:N_PTS] = pts

    in_maps = []
    for c in range(N_CORES):
        m = {"pts": pts_pad[c * PTS_PER_CORE : (c + 1) * PTS_PER_CORE]}
        for i in range(len(LODS)):
            m[f"tab{i}"] = tabs[i]
        in_maps.append(m)

    res = run_bass_kernel_spmd(nc, in_maps, core_ids=list(range(N_CORES)))
    global _last_results
    _last_results = res
    out = np.concatenate([r["out"] for r in res.results], axis=0)[:N_PTS]
    return np.ascontiguousarray(out, dtype=np.float32)


_last_results = None



# revision 5
# speedup vs baseline: 1.2501x; 1.2501x over previous
"""Multi-resolution dense-grid trilinear interpolation (embedding lookup).

Strategy:
  - Host: pack each LOD grid into a "cell table" tab[cell, 0:8] = the 8 corner
    values of the cell whose base corner has flat index `cell` (flat-index
    aliasing + clip matches jnp's OOB clamp semantics exactly).
  - Shard points across 8 NeuronCores (data parallel, tables replicated).
  - Device per core: for each point batch and each level, compute
    xs = pts*(res-1)/2 + (res-1)/2, frac f = xs mod 1, floor = xs - f,
    cell = (cz*res + cy)*res + cx (exact in f32, < 2^24), then gather the 32B
    cell row with one indirect-DMA descriptor per point, and evaluate the
    trilinear lerp chain on the vector engine. Sum over 5 levels.

Scheduling note: the walrus build in this toolchain only allows ONE sync-wait
command per instruction, so the kernel is structured so Tile never needs two:
cross-engine tiles (idx) get unique slots (no recycle waits), pts/acc are
single tiles loaded/stored once, and each gather is preceded by a tiny Pool
"+0" probe on idx so the idx-ready (DVE) wait lands on the probe while the
gather only carries the cells-slot-recycle (DMASW) wait.
"""

import numpy as np

import concourse.bass as bass
import concourse.mybir as mybir
import concourse.tile as tile
from concourse.bass import IndirectOffsetOnAxis
from concourse.bass_utils import run_bass_kernel_spmd
from concourse.tile import add_dep_helper

LODS = [16, 32, 64, 128, 256]
# L0 (16-grid) nests exactly into L4 (256-grid): 255 = 15*17, so every L0
# breakpoint j/15 = 17j/255 is an L4 node. Folding trilerp_0 sampled at the
# L4 nodes into cb4 makes trilerp4(cb4') == trilerp4(cb4) + trilerp0(cb0)
# exactly (trilinear interpolants that agree on all 8 corners of each fine
# cell are identical). The device then runs 4 levels instead of 5.
DEV_LODS = [32, 64, 128, 256]
N_PTS = 2_000_000
N_CORES = 8
P = 128

BATCH_FREE = 490  # points per partition per batch
N_BATCHES = 4
PTS_PER_CORE = P * BATCH_FREE * N_BATCHES  # 250_880 (>= 250_000)

F32 = mybir.dt.float32
I32 = mybir.dt.int32
AF = mybir.ActivationFunctionType
ALU = mybir.AluOpType


def pack_cell_table(cb: np.ndarray, res: int) -> np.ndarray:
    """tab[i, dz*4+dy*2+dx] = cb_flat[min(i + dx + dy*res + dz*res^2, n-1)]."""
    flat = np.ascontiguousarray(np.asarray(cb, dtype=np.float32).reshape(-1))
    n = flat.shape[0]
    tab = np.empty((n, 8), dtype=np.float32)
    base = np.arange(n, dtype=np.int64)
    c = 0
    for dz in (0, 1):
        for dy in (0, 1):
            for dx in (0, 1):
                off = dx + dy * res + dz * res * res
                if off == 0:
                    tab[:, c] = flat
                else:
                    idx = base + off
                    np.minimum(idx, n - 1, out=idx)
                    tab[:, c] = flat[idx]
                c += 1
    return tab


def build_module(batch_free: int = BATCH_FREE, n_batches: int = N_BATCHES):
    """Build the single-core Bass program (run SPMD on all 8 cores)."""
    n = batch_free
    npts = P * n * n_batches
    ntot = n * n_batches  # free elems per partition overall
    nc = bass.Bass("TRN2", target_bir_lowering=False, debug=False)

    pts = nc.dram_tensor("pts", [npts, 3], F32, kind="ExternalInput").ap()
    tabs = [
        nc.dram_tensor(f"tab{i}", [r**3, 8], F32, kind="ExternalInput").ap()
        for i, r in enumerate(DEV_LODS)
    ]
    out = nc.dram_tensor("out", [npts, 1], F32, kind="ExternalOutput").ap()

    with tile.TileContext(nc) as tc:
        with (
            tc.tile_pool(name="p_io", bufs=1) as io_pool,
            tc.tile_pool(name="p_coord", bufs=2) as coord_pool,
            tc.tile_pool(name="p_idx", bufs=1) as idx_pool,
            tc.tile_pool(name="p_cells", bufs=3) as cells_pool,
            tc.tile_pool(name="p_lerp", bufs=2) as lerp_pool,
        ):
            # one DMA for all points: [P, ntot, 3], per-partition contiguous
            pts_all = io_pool.tile([P, ntot, 3], F32, name="pts_all")
            pts_load = nc.sync.dma_start(
                out=pts_all[:],
                in_=pts[:, :].rearrange("(p n) t -> p n t", p=P),
            )
            acc_all = io_pool.tile([P, ntot], F32, name="acc_all")
            pool_scratch = io_pool.tile(
                [P, n_batches * len(DEV_LODS)], I32, name="pool_scratch"
            )
            dve_obs = io_pool.tile(
                [1, n_batches * len(DEV_LODS)], I32, name="dve_obs"
            )
            obs_f32 = io_pool.tile([1, 8], F32, name="obs_f32")
            pool_f32 = io_pool.tile(
                [1, 8 * n_batches * len(DEV_LODS)], F32, name="pool_f32"
            )
            probe_slot = 0
            pf_slot = 0
            last_col_gathers = []
            probe_insts = []
            cells_readers = []
            pool_obs_hist = []
            CELLS_BUFS = 3

            for b in range(n_batches):
                pts_b = pts_all[:, b * n : (b + 1) * n, :]
                acc_b = acc_all[:, b * n : (b + 1) * n]

                for li, res in enumerate(DEV_LODS):
                    s = (res - 1) / 2.0
                    # xs = pts*s + s
                    xs = coord_pool.tile([P, n, 3], F32, name="xs", tag="xs")
                    nc.vector.tensor_scalar(
                        out=xs[:], in0=pts_b, scalar1=s, scalar2=s,
                        op0=ALU.mult, op1=ALU.add,
                    )
                    # floor via int32 trunc-cast roundtrip (xs >= 0), f = xs - floor
                    ci = coord_pool.tile([P, n, 3], I32, name="ci", tag="ci")
                    nc.vector.tensor_copy(out=ci[:], in_=xs[:])
                    flo = coord_pool.tile([P, n, 3], F32, name="flo", tag="flo")
                    nc.vector.tensor_copy(out=flo[:], in_=ci[:])
                    # robust to the DVE converter's rounding mode: if the
                    # cast rounded up (flo > xs), subtract 1 to get floor.
                    # f doubles as scratch for the correction mask.
                    f = coord_pool.tile([P, n, 3], F32, name="f", tag="f")
                    nc.vector.tensor_tensor(
                        out=f[:], in0=flo[:], in1=xs[:], op=ALU.is_gt
                    )
                    nc.vector.tensor_tensor(
                        out=flo[:], in0=flo[:], in1=f[:], op=ALU.subtract
                    )
                    nc.vector.tensor_tensor(
                        out=f[:], in0=xs[:], in1=flo[:], op=ALU.subtract
                    )
                    # cell = (cz*res + cy)*res + cx  (f32 exact, < 2^24)
                    idx_f = coord_pool.tile([P, n], F32, name="idx_f", tag="idx_f")
                    nc.vector.scalar_tensor_tensor(
                        out=idx_f[:], in0=flo[:, :, 2], scalar=float(res),
                        in1=flo[:, :, 1], op0=ALU.mult, op1=ALU.add,
                    )
                    nc.vector.scalar_tensor_tensor(
                        out=idx_f[:], in0=idx_f[:], scalar=float(res),
                        in1=flo[:, :, 0], op0=ALU.mult, op1=ALU.add,
                    )
                    # own slot per (batch, level): no recycle waits
                    idx = idx_pool.tile(
                        [P, n], I32, name=f"idx_{b}_{li}", tag=f"idx_{b}_{li}"
                    )
                    nc.vector.tensor_copy(out=idx[:], in_=idx_f[:])
                    # Pool-engine probe: reads one idx element into a scratch
                    # tile. The probe absorbs the DVE idx-ready wait into the
                    # Pool sequencer clock; the gathers are ordered after it
                    # with sync-free deps, so they need no wait of their own
                    # for idx (walrus here fits only one sync-wait command
                    # per instruction).
                    probe = nc.gpsimd.tensor_scalar(
                        out=pool_scratch[0:1, probe_slot : probe_slot + 1],
                        in0=idx[0:1, 0:1],
                        scalar1=0, scalar2=None, op0=ALU.add,
                    )
                    probe_slot += 1
                    if len(cells_readers) >= CELLS_BUFS:
                        # merge the recycled slot's last-reader tick into the
                        # probe's DVE wait (same sem -> still one condition)
                        add_dep_helper(
                            probe.ins, cells_readers[-CELLS_BUFS].ins, sync=True,
                            reason="probe covers recycled cells slot readers",
                        )
                        # order after the evicted tenant's Pool lane observers
                        # so the first gather's writer-waits are elided
                        for pob in pool_obs_hist[-CELLS_BUFS]:
                            add_dep_helper(
                                probe.ins, pob.ins, sync=False,
                                reason="probe after prev tenant lane observers",
                            )

                    # gather 8-f32 cell rows, one point-column per instruction
                    # (HW only supports one dynamic offset per partition):
                    # cells[p, i, :] = tab[idx[p, i], :]
                    cells = cells_pool.tile([P, n, 8], F32, name="cells")
                    col_gathers = []
                    for i in range(n):
                        g = nc.gpsimd.indirect_dma_start(
                            out=cells[:, i, :],
                            out_offset=None,
                            in_=tabs[li][:],
                            in_offset=IndirectOffsetOnAxis(
                                ap=idx[:, i : i + 1], axis=0
                            ),
                        )
                        add_dep_helper(
                            g.ins, probe.ins, sync=False,
                            reason="gather after idx probe (pool seq order)",
                        )
                        col_gathers.append(g)
                    last_col_gathers = col_gathers
                    probe_insts.append(probe)
                    # DVE observers: one tiny copy per DMASW lane (the last 8
                    # gathers cover all lanes round-robin), so the lerp's
                    # cells-read needs no multi-lane wait of its own. Pool
                    # observers do the same for the Pool sequencer clock so
                    # the slot's next tenant needs no multi-lane wait either.
                    observers = []
                    pool_obs = []
                    for k in range(min(8, n)):
                        col = n - 1 - k
                        ob = nc.vector.tensor_copy(
                            out=obs_f32[0:1, k : k + 1],
                            in_=cells[0:1, col, 0:1],
                        )
                        observers.append(ob)
                        pob = nc.gpsimd.tensor_scalar(
                            out=pool_f32[0:1, pf_slot : pf_slot + 1],
                            in0=cells[0:1, col, 0:1],
                            scalar1=0.0, scalar2=None, op0=ALU.add,
                        )
                        pf_slot += 1
                        pool_obs.append(pob)
                    pool_obs_hist.append(pool_obs)

                    # trilinear lerp chain: 8 -> 4 -> 2 -> 1 (in-place)
                    cr = cells[:].rearrange("p n (j two) -> p n j two", two=2)
                    d1 = lerp_pool.tile([P, n, 4], F32, name="d1", tag="d1")
                    d1_sub = nc.vector.tensor_tensor(
                        out=d1[:], in0=cr[:, :, :, 1], in1=cr[:, :, :, 0],
                        op=ALU.subtract,
                    )
                    for ob in observers:
                        add_dep_helper(
                            d1_sub.ins, ob.ins, sync=False,
                            reason="lerp after lane observers (dve order)",
                        )
                    fx = f[:, :, 0:1].to_broadcast([P, n, 4])
                    nc.vector.tensor_tensor(out=d1[:], in0=d1[:], in1=fx, op=ALU.mult)
                    t1_add = nc.vector.tensor_tensor(
                        out=d1[:], in0=cr[:, :, :, 0], in1=d1[:], op=ALU.add
                    )
                    cells_readers.append(t1_add)

                    tr = d1[:].rearrange("p n (j two) -> p n j two", two=2)
                    d2 = lerp_pool.tile([P, n, 2], F32, name="d2", tag="d2")
                    nc.vector.tensor_tensor(
                        out=d2[:], in0=tr[:, :, :, 1], in1=tr[:, :, :, 0],
                        op=ALU.subtract,
                    )
                    fy = f[:, :, 1:2].to_broadcast([P, n, 2])
                    nc.vector.tensor_tensor(out=d2[:], in0=d2[:], in1=fy, op=ALU.mult)
                    nc.vector.tensor_tensor(
                        out=d2[:], in0=tr[:, :, :, 0], in1=d2[:], op=ALU.add
                    )

                    d3 = lerp_pool.tile([P, n], F32, name="d3", tag="d3")
                    nc.vector.tensor_tensor(
                        out=d3[:], in0=d2[:, :, 1], in1=d2[:, :, 0], op=ALU.subtract
                    )
                    nc.vector.tensor_tensor(
                        out=d3[:], in0=d3[:], in1=f[:, :, 2], op=ALU.mult
                    )
                    if li == 0:
                        nc.vector.tensor_tensor(
                            out=acc_b, in0=d2[:, :, 0], in1=d3[:], op=ALU.add
                        )
                        final_acc = None
                    else:
                        c3 = lerp_pool.tile([P, n], F32, name="c3", tag="c3")
                        nc.vector.tensor_tensor(
                            out=c3[:], in0=d2[:, :, 0], in1=d3[:], op=ALU.add
                        )
                        final_acc = nc.vector.tensor_tensor(
                            out=acc_b, in0=acc_b, in1=c3[:], op=ALU.add
                        )

            # ---- drain-tail flattening (1 sync-wait per instruction limit) --
            # DVE observes the Pool clock by reading all probe outputs; force
            # it before the final acc so the out-DMA's DVE wait covers it.
            probe_obs = nc.vector.tensor_copy(
                out=dve_obs[:], in_=pool_scratch[0:1, :]
            )
            assert final_acc is not None
            add_dep_helper(
                final_acc.ins, probe_obs.ins, sync=False,
                reason="probe observation before final acc",
            )
            # SP observes every proc's final tick directly, one nop (= one
            # sync-wait) per proc, so the tail drain ends up with a single
            # remaining wait (the out-DMA's own lane).
            obs_targets = [pts_load, probe_insts[-1], probe_obs, final_acc]
            obs_targets += pool_obs_hist[-1]
            obs_targets += last_col_gathers[-8:]
            for k, tgt in enumerate(obs_targets):
                sp_nop = nc.sync.nop(nofuse=True, hint=f"obs_{k}")
                add_dep_helper(
                    sp_nop.ins, tgt.ins, sync=True,
                    reason="SP observes proc completion before drain",
                )

            nc.sync.dma_start(
                out=out[:, :].rearrange("(p n) o -> p (n o)", p=P),
                in_=acc_all[:],
            )

    return nc


_MODULE_CACHE = {}


def _get_module():
    key = (BATCH_FREE, N_BATCHES)
    if key not in _MODULE_CACHE:
        _MODULE_CACHE[key] = build_module()
    return _MODULE_CACHE[key]


def _fold_l0_into_l4(cb0: np.ndarray, cb4: np.ndarray) -> np.ndarray:
    """cb4'[node] = cb4[node] + trilerp_0(node position), node = L4 grid.

    L4 node i maps to L0 coordinate i*15/255 = i/17: floor q = i//17,
    frac w = (i % 17)/17 — exact rationals, so this is the exact sampling
    of the reference's level-0 interpolant at the level-4 nodes.
    """
    C = np.asarray(cb0, dtype=np.float32).reshape(16, 16, 16)  # [cz, cy, cx]
    i = np.arange(256)
    q = np.minimum(i // 17, 15)
    q1 = np.minimum(q + 1, 15)
    w = ((i % 17) / 17.0).astype(np.float32)
    # z-interp -> [256, 16, 16]
    t1 = C[q] * (1 - w)[:, None, None] + C[q1] * w[:, None, None]
    # y-interp -> [256, 256, 16]
    t2 = t1[:, q] * (1 - w)[None, :, None] + t1[:, q1] * w[None, :, None]
    # x-interp -> [256, 256, 256]
    t3 = t2[:, :, q] * (1 - w)[None, None, :] + t2[:, :, q1] * w[None, None, :]
    return np.asarray(cb4, dtype=np.float32).reshape(256, 256, 256) + t3


def kernel(pts, cb0, cb1, cb2, cb3, cb4):
    nc = _get_module()
    cb4m = _fold_l0_into_l4(cb0, cb4)
    cbs = [cb1, cb2, cb3, cb4m]
    tabs = [pack_cell_table(cb, r) for cb, r in zip(cbs, DEV_LODS)]

    pts = np.ascontiguousarray(np.asarray(pts, dtype=np.float32))
    assert pts.shape == (N_PTS, 3)
    pts_pad = np.zeros((N_CORES * PTS_PER_CORE, 3), dtype=np.float32)
    pts_pad[:N_PTS] = pts

    in_maps = []
    for c in range(N_CORES):
        m = {"pts": pts_pad[c * PTS_PER_CORE : (c + 1) * PTS_PER_CORE]}
        for i in range(len(DEV_LODS)):
            m[f"tab{i}"] = tabs[i]
        in_maps.append(m)

    res = run_bass_kernel_spmd(nc, in_maps, core_ids=list(range(N_CORES)))
    global _last_results
    _last_results = res
    out = np.concatenate([r["out"] for r in res.results], axis=0)[:N_PTS]
    return np.ascontiguousarray(out, dtype=np.float32)


_last_results = None

